# revision 33
# baseline (speedup 1.0000x reference)
"""Affinity-propagation spatial stencil kernel for Trainium2 (8 NeuronCores).

Data-parallel: 16 images sharded 2-per-core; a core's 2 images are merged
into the free dimension as 8 flattened rows-per-partition ([P, 8, W]:
rows 4b..4b+3 belong to image b), so every engine op uses a 2-level
free access pattern (the DVE 2x fp16 mode and the GPSIMD ucode both
degrade on deeper APs).

Math (A_k = zero-padded shift by OFFSETS[k]; G_k = guidance channel k):
  absw = sum_k A_k |G_k|;  inv = 1/absw = exp(-ln(absw))
  gate_sum = (sum_k A_k G_k) * inv;  bias = raw - gate_sum * raw
  step:  r' = inv * (sum_k A_k (G_k * r)) + bias
(A_k G_k)*(A_k r) = A_k (G_k * r): products are unshifted muls; only the
shift-SUM moves data.  Column shifts ride free-dim AP offsets (guard
columns); row shifts act within a partition's 4 rows except the
partition-crossing row, which the idle TensorEngine produces as a matmul
with a sub/super-diagonal 0/1 stationary into PSUM (halo_dn[m] =
up[m+1, row0]).  ACT drains PSUM to SBUF fp16 (DVE reading PSUM directly
measured ~10x slow); the edge adds then run on fp16 in SBUF.

Per-step engine split: DVE products g0..g6 + up/u0 trees + combine +
inv/bias; GPSIMD the g7 product + um tree + one combine row-add; ACT the
2 PSUM drains; PE 4 halo matmuls.  Setup streams loads through 3
rotating stage buffers (depth-3 pipeline ~ HBM bound), converts on
ACT (k<4) / DVE (k>=4), abs on ACT, and runs the absw and gate-sum trees
behind the loads.
"""

import sys

sys.path.insert(0, "/opt/trn_rl_repo")

import numpy as np

import concourse.bass as bass
import concourse.mybir as mybir
from concourse import tile
from concourse.bass_utils import run_bass_kernel_spmd

N_CORES = 8
B, K, H, W = 16, 8, 512, 512
BPC = B // N_CORES  # images per core (merged: 8 rows per partition)
P = 128
RPP = H // P  # rows per partition per image
R2 = BPC * RPP  # flattened rows per partition
WG = W + 4  # guarded row width (image cols at [2:514])
C0 = 2
PROP_TIME = 4

F32 = mybir.dt.float32
DT = mybir.dt.float16
AT = mybir.AluOpType
AF = mybir.ActivationFunctionType


def _split_excess_waits(nc):
    """This walrus build encodes at most 1 sem wait per instruction; move the
    overflow onto preceding NoOps. Also drop EVENT_SEMAPHORE_RANGE_CLEAR
    (unencodable here; only appears at the kernel tail where it's a no-op)."""
    for f in nc.m.functions:
        for bb in f.blocks:
            new_insts = []
            for ins in bb.instructions:
                if getattr(ins, "op_name", None) == "EVENT_SEMAPHORE_RANGE_CLEAR":
                    continue
                cap = 1
                si = getattr(ins, "sync_info", None)
                if si is not None and si.on_wait and len(si.on_wait) > cap:
                    extra = list(si.on_wait[cap:])
                    del si.on_wait[cap:]
                    while extra:
                        nop = mybir.InstNoOp(
                            name=nc.get_next_instruction_name(),
                            engine=ins.engine,
                            sync_info=mybir.SyncInfo(on_wait=extra[:cap], on_update=[]),
                        )
                        new_insts.append(nop)
                        extra = extra[cap:]
                new_insts.append(ins)
            bb.instructions[:] = new_insts


def _c(ap):
    """center (image) view of a guarded [P, R2, WG] tile."""
    return ap[:, :, C0 : C0 + W]


def _w(ap, dj):
    """column-shifted view of a guarded tile: value at [i, j+dj]."""
    return ap[:, :, C0 + dj : C0 + dj + W]


def _emit(nc, pool, psum, g_dram, d_dram, o_dram):
    V = nc.vector
    GP = nc.gpsimd
    ACT = nc.scalar
    PE = nc.tensor

    def gtile(name):  # guarded work tile
        return pool.tile([P, R2, WG], DT, name=name)

    def utile(name):  # unguarded work tile
        return pool.tile([P, R2, W], DT, name=name)

    gates = pool.tile([P, K, R2, WG], DT, name="gates")
    stages = [pool.tile([P, RPP, W], F32, name=f"stg{i}") for i in range(4)]
    # sA/sB: abs ping-pong (setup); tmp1/tmp2: product temps (step-1 stream
    # during the load window, then every prop step -- each product is
    # consumed by the immediately following tree add, so two suffice).
    sA, sB = gtile("sA"), gtile("sB")
    tmp1, tmp2 = gtile("tmp1"), gtile("tmp2")
    # A-set: absw trees (streamed), then gate-sum trees, then step 2..4 trees.
    upA, u0A, umA = utile("upA"), utile("u0A"), utile("umA")
    # C-set: step-1 class sums (streamed during loads); step 2..4 combine dest.
    upC, u0C, umC = utile("upC"), utile("u0C"), utile("umC")
    rA = gtile("rA")
    inv = utile("inv")
    bias = utile("bias")
    # shift matrices for the PE halo: halo_dn[m]=x[m+1], halo_up[m]=x[m-1];
    # wid = identity (accumulates in-partition rows into the same PSUM bank)
    wdn = pool.tile([P, P], DT, name="wdn")
    wup = pool.tile([P, P], DT, name="wup")
    wid = pool.tile([P, P], DT, name="wid")
    ci = pool.tile([P, P], F32, name="ci")
    pm1 = pool.tile([P, 1], F32, name="pm1")
    pp1 = pool.tile([P, 1], F32, name="pp1")
    pz0 = pool.tile([P, 1], F32, name="pz0")
    psum_dn = psum.tile([P, BPC, W], F32, name="psum_dn", bufs=2)
    psum_up = psum.tile([P, BPC, W], F32, name="psum_up", bufs=2)

    def gv(k, dj=0):  # column-shifted gate view [P, R2, W]
        return gates[:, k, :, C0 + dj : C0 + dj + W]

    # ---- loads: depth-4 pipeline over 4 rotating stage buffers; the first
    # triggers are emitted before any constants so DMA starts immediately ----
    load_i = 0

    def load(dram_plane, cast_eng, dst, halves=1):
        # halves=2 halves the DMA landing latency for the kernel lead-in,
        # where the very first casts gate everything downstream.
        nonlocal load_i
        st = stages[load_i % 4]
        load_i += 1
        src = dram_plane.rearrange("(p r) j -> p r j", p=P)
        hr = RPP // halves
        for h in range(halves):
            rs = slice(h * hr, (h + 1) * hr)
            nc.sync.dma_start(out=st[:, rs, :], in_=src[:, rs, :])
            if cast_eng is V:
                V.tensor_copy(dst[:, rs, :], st[:, rs, :])
            else:
                ACT.activation(dst[:, rs, :], st[:, rs, :], AF.Copy)

    for b in range(BPC):
        load(d_dram[b, 0], V, rA[:, 4 * b : 4 * b + 4, C0 : C0 + W], halves=2)

    # ---- constants: shift matrices via iota + is_equal ----
    GP.iota(ci[:], [[1, P]], base=0, channel_multiplier=0,
            allow_small_or_imprecise_dtypes=True)  # ci[p,j] = j
    GP.iota(pm1[:], [[1, 1]], base=-1, channel_multiplier=1,
            allow_small_or_imprecise_dtypes=True)  # p-1
    GP.iota(pp1[:], [[1, 1]], base=1, channel_multiplier=1,
            allow_small_or_imprecise_dtypes=True)  # p+1
    GP.iota(pz0[:], [[1, 1]], base=0, channel_multiplier=1,
            allow_small_or_imprecise_dtypes=True)  # p
    # wdn[p,m] = 1 iff p == m+1  <=>  m == p-1 ; wup[p,m] = 1 iff m == p+1
    V.tensor_scalar(wdn[:], ci[:], pm1[:, 0:1], None, AT.is_equal)
    V.tensor_scalar(wup[:], ci[:], pp1[:, 0:1], None, AT.is_equal)
    V.tensor_scalar(wid[:], ci[:], pz0[:, 0:1], None, AT.is_equal)

    # ---- zero guard columns (written once; ops below write centers only) ----
    GP.memset(gates[:, :, :, 0:C0], 0.0)
    GP.memset(gates[:, :, :, C0 + W : WG], 0.0)
    for t in (sA, sB, tmp1, tmp2, rA):
        GP.memset(t[:, :, 0:C0], 0.0)
        GP.memset(t[:, :, C0 + W : WG], 0.0)

    # abs ping-pong: even k -> sA, odd k -> sB (product scratches, free here)
    def _abs(k):
        dst = sA if k % 2 == 0 else sB
        ACT.activation(_c(dst), gv(k), AF.Abs)
        return dst

    # Streamed per gate k: loads + cast, |g_k| (ACT), step-1 product
    # g_k * raw (DVE, into tmp1/tmp2), and both the absw tree (A-set) and
    # step-1 tree (C-set) as their operands complete.
    for k in range(K):
        eng = ACT if k < 4 else V
        for b in range(BPC):
            load(g_dram[b, k], eng, gates[:, k, 4 * b : 4 * b + 4, C0 : C0 + W],
                 halves=2 if k == 0 else 1)
        _abs(k)
        t = tmp1 if k % 2 == 0 else tmp2
        V.tensor_mul(_c(t), gv(k), _c(rA))
        if k == 1:
            V.tensor_add(upC[:], _w(tmp1, 1), _c(tmp2))
            V.tensor_add(upA[:], _w(sA, 1), _c(sB))
        elif k == 2:
            V.tensor_add(upC[:], upC[:], _w(tmp1, -1))
            V.tensor_add(upA[:], upA[:], _w(sA, -1))
        elif k == 4:
            V.tensor_add(u0C[:], _w(tmp2, 1), _w(tmp1, -1))
            V.tensor_add(u0A[:], _w(sB, 1), _w(sA, -1))
        elif k == 6:
            V.tensor_add(umC[:], _w(tmp2, 1), _c(tmp1))
            V.tensor_add(umA[:], _w(sB, 1), _c(sA))
        elif k == 7:
            V.tensor_add(umC[:], umC[:], _w(tmp2, -1))
            V.tensor_add(umA[:], umA[:], _w(sB, -1))

    def combine(dst, up_t, u0_t, um_t, u0_after_um=False):
        """dst[q] = up[q+1] + u0[q] + um[q-1] per image (q in 0..3).
        u0_t: a plane, or a list of fns mapping a row slice to a
        column-shifted view (the u0 class folded into the PSUM groups and
        interior adds without materializing its own tree sum).

        Partition-edge rows (q=0 and q=3) are built ENTIRELY in PSUM: the
        cross-partition halo term via the wdn/wup shift matmul plus the two
        in-partition terms via identity matmuls accumulating into the same
        bank; ACT drains then write those rows of dst directly.  DVE adds
        only the interior rows (q=1,2).  GPSIMD is never used: its tensor
        ucode both runs ~3x below DVE and starves DVE's SBUF ports ~4x
        while active (measured), so everything elementwise stays on DVE.
        """
        u0_fns = u0_t if isinstance(u0_t, list) else [lambda rs, t=u0_t: t[:, rs, :]]
        for b in range(BPC):
            mms = [(wdn, up_t[:, 4 * b, :])]
            mms += [(wid, f(4 * b + 3)) for f in u0_fns]
            mms += [(wid, um_t[:, 4 * b + 2, :])]
            if u0_after_um:  # u0 operands finish last: keep them at the tail
                mms = [mms[0], mms[-1]] + mms[1:-1]
            for i, (wm, rhs) in enumerate(mms):
                PE.matmul(out=psum_dn[:, b, :], lhsT=wm[:], rhs=rhs,
                          start=(i == 0), stop=(i == len(mms) - 1))
            mms = [(wid, up_t[:, 4 * b + 1, :])]
            mms += [(wid, f(4 * b)) for f in u0_fns]
            mms += [(wup, um_t[:, 4 * b + 3, :])]
            if u0_after_um:
                mms = [mms[0], mms[-1]] + mms[1:-1]
            for i, (wm, rhs) in enumerate(mms):
                PE.matmul(out=psum_up[:, b, :], lhsT=wm[:], rhs=rhs,
                          start=(i == 0), stop=(i == len(mms) - 1))
        ACT.activation(dst[:, 3::RPP, :], psum_dn[:], AF.Copy)
        ACT.activation(dst[:, 0::RPP, :], psum_up[:], AF.Copy)
        for b in range(BPC):
            V.tensor_add(dst[:, 4 * b + 1 : 4 * b + 3, :],
                         up_t[:, 4 * b + 2 : 4 * b + 4, :],
                         u0_fns[0](slice(4 * b + 1, 4 * b + 3)))
            for f in u0_fns[1:]:
                V.tensor_add(dst[:, 4 * b + 1 : 4 * b + 3, :],
                             dst[:, 4 * b + 1 : 4 * b + 3, :],
                             f(slice(4 * b + 1, 4 * b + 3)))
        for b in range(BPC):
            V.tensor_add(dst[:, 4 * b + 1 : 4 * b + 3, :],
                         dst[:, 4 * b + 1 : 4 * b + 3, :],
                         um_t[:, 4 * b : 4 * b + 2, :])

    # ---- absw -> inv = exp(-ln(absw)); combine lands in `bias` scratch ----
    combine(bias, upA, u0A, umA)
    for b in range(BPC):
        ACT.activation(stages[b][:], bias[:, 4 * b : 4 * b + 4, :], AF.Ln)
        ACT.activation(inv[:, 4 * b : 4 * b + 4, :], stages[b][:], AF.Exp, scale=-1.0)

    # ---- step-1 shift-sum (inputs streamed above); lands in tmp1 center ----
    combine(_c(tmp1), upC, u0C, umC)

    # ---- gate_sum (A-set reused) -> bias = raw - gate_sum*inv*raw;
    # the u0 class reads the (guarded) gate planes directly ----
    V.tensor_add(upA[:], gv(0, 1), gv(1))
    V.tensor_add(upA[:], upA[:], gv(2, -1))
    V.tensor_add(umA[:], gv(5, 1), gv(6))
    V.tensor_add(umA[:], umA[:], gv(7, -1))

    def gvr(k, dj):
        return lambda rs: gates[:, k, rs, C0 + dj : C0 + dj + W]

    def tvr(t, dj):
        return lambda rs: t[:, rs, C0 + dj : C0 + dj + W]

    combine(_c(tmp2), upA, [gvr(3, 1), gvr(4, -1)], umA)
    V.tensor_mul(_c(tmp2), _c(tmp2), _c(rA))  # gate_sum_unnorm * raw
    V.tensor_mul(_c(tmp2), _c(tmp2), inv[:])
    V.tensor_sub(bias[:], _c(rA), _c(tmp2))

    # ---- finish step 1: r1 = inv * U1 + bias ----
    V.tensor_mul(_c(tmp1), _c(tmp1), inv[:])
    V.tensor_add(_c(rA), _c(tmp1), bias[:])

    def norm_split(acc, last):
        """r = inv*acc + bias, interior rows first (edge rows arrive late
        via PE+drain), then the stepped edge rows."""
        for b in range(BPC):
            V.tensor_mul(acc[:, 4 * b + 1 : 4 * b + 3, :],
                         acc[:, 4 * b + 1 : 4 * b + 3, :],
                         inv[:, 4 * b + 1 : 4 * b + 3, :])
            V.tensor_add(rA[:, 4 * b + 1 : 4 * b + 3, C0 : C0 + W],
                         acc[:, 4 * b + 1 : 4 * b + 3, :],
                         bias[:, 4 * b + 1 : 4 * b + 3, :])
        for q0 in (3, 0):
            V.tensor_mul(acc[:, q0::RPP, :], acc[:, q0::RPP, :], inv[:, q0::RPP, :])
            V.tensor_add(rA[:, q0::RPP, C0 : C0 + W], acc[:, q0::RPP, :],
                         bias[:, q0::RPP, :])
        if last:
            for b in range(BPC):
                st = stages[b]
                V.tensor_copy(st[:], rA[:, 4 * b : 4 * b + 4, C0 : C0 + W])
                nc.sync.dma_start(
                    out=o_dram[b, 0].rearrange("(p r) j -> p r j", p=P), in_=st[:]
                )

    def norm_split_last(acc):
        """Final step: finish image b completely, then cast+store it while
        the other image's rows are still being normalized."""
        for b in range(BPC):
            V.tensor_mul(acc[:, 4 * b + 1 : 4 * b + 3, :],
                         acc[:, 4 * b + 1 : 4 * b + 3, :],
                         inv[:, 4 * b + 1 : 4 * b + 3, :])
            V.tensor_add(rA[:, 4 * b + 1 : 4 * b + 3, C0 : C0 + W],
                         acc[:, 4 * b + 1 : 4 * b + 3, :],
                         bias[:, 4 * b + 1 : 4 * b + 3, :])
            for q in (4 * b + 3, 4 * b):
                V.tensor_mul(acc[:, q, :], acc[:, q, :], inv[:, q, :])
                V.tensor_add(rA[:, q, C0 : C0 + W], acc[:, q, :], bias[:, q, :])
            st = stages[b]
            V.tensor_copy(st[:], rA[:, 4 * b : 4 * b + 4, C0 : C0 + W])
            nc.sync.dma_start(
                out=o_dram[b, 0].rearrange("(p r) j -> p r j", p=P), in_=st[:]
            )

    # ---- steps 2..4, r updated in place in rA ----
    # Class order up, um, then the two u0 products LAST (p3 -> tmp2, p4 ->
    # tmp1, both persisting through the combine): the u0 class never
    # materializes a tree sum -- its two column-shifted product planes feed
    # the PSUM groups and interior adds directly.
    for step in range(1, PROP_TIME):
        V.tensor_mul(_c(tmp1), gv(0), _c(rA))
        V.tensor_mul(_c(tmp2), gv(1), _c(rA))
        V.tensor_add(upA[:], _w(tmp1, 1), _c(tmp2))
        V.tensor_mul(_c(tmp1), gv(2), _c(rA))
        V.tensor_add(upA[:], upA[:], _w(tmp1, -1))
        V.tensor_mul(_c(tmp1), gv(5), _c(rA))
        V.tensor_mul(_c(tmp2), gv(6), _c(rA))
        V.tensor_add(umA[:], _w(tmp1, 1), _c(tmp2))
        V.tensor_mul(_c(tmp1), gv(7), _c(rA))
        V.tensor_add(umA[:], umA[:], _w(tmp1, -1))
        V.tensor_mul(_c(tmp2), gv(3), _c(rA))
        V.tensor_mul(_c(tmp1), gv(4), _c(rA))
        combine(upC, upA, [tvr(tmp2, 1), tvr(tmp1, -1)], umA, u0_after_um=True)
        if step < PROP_TIME - 1:
            norm_split(upC, last=False)
        else:
            norm_split_last(upC)


def build(legalize=True):
    nc = bass.Bass()
    g_dram = nc.declare_dram_parameter("guidance", [BPC, K, H, W], F32, isOutput=False)
    d_dram = nc.declare_dram_parameter("blur_depth", [BPC, 1, H, W], F32, isOutput=False)
    o_dram = nc.declare_dram_parameter("out", [BPC, 1, H, W], F32, isOutput=True)
    with tile.TileContext(nc) as tc:
        with tc.tile_pool(name="main", bufs=1) as pool:
            with tc.tile_pool(name="ps", space="PSUM", bufs=1) as psum:
                _emit(nc, pool, psum, g_dram, d_dram, o_dram)
    if legalize:
        _split_excess_waits(nc)
    return nc


_NC = None


def _get_nc():
    global _NC
    if _NC is None:
        _NC = build()
    return _NC


def run(guidance, blur_depth, **spmd_kwargs):
    nc = _get_nc()
    in_maps = [
        {
            "guidance": np.ascontiguousarray(guidance[BPC * c : BPC * (c + 1)]),
            "blur_depth": np.ascontiguousarray(blur_depth[BPC * c : BPC * (c + 1)]),
        }
        for c in range(N_CORES)
    ]
    res = run_bass_kernel_spmd(nc, in_maps, list(range(N_CORES)), **spmd_kwargs)
    out = np.concatenate([res.results[i]["out"] for i in range(N_CORES)], axis=0)
    return out, res


def kernel(guidance, blur_depth):
    out, _ = run(guidance, blur_depth)
    return out.astype(np.float32)


# revision 37
# speedup vs baseline: 1.0138x; 1.0138x over previous
"""Affinity-propagation spatial stencil kernel for Trainium2 (8 NeuronCores).

Data-parallel: 16 images sharded 2-per-core; a core's 2 images are merged
into the free dimension as 8 flattened rows-per-partition ([P, 8, W]:
rows 4b..4b+3 belong to image b), so every engine op uses a 2-level
free access pattern (the DVE 2x fp16 mode and the GPSIMD ucode both
degrade on deeper APs).

Math (A_k = zero-padded shift by OFFSETS[k]; G_k = guidance channel k):
  absw = sum_k A_k |G_k|;  inv = 1/absw = exp(-ln(absw))
  gate_sum = (sum_k A_k G_k) * inv;  bias = raw - gate_sum * raw
  step:  r' = inv * (sum_k A_k (G_k * r)) + bias
(A_k G_k)*(A_k r) = A_k (G_k * r): products are unshifted muls; only the
shift-SUM moves data.  Column shifts ride free-dim AP offsets (guard
columns); row shifts act within a partition's 4 rows except the
partition-crossing row, which the idle TensorEngine produces as a matmul
with a sub/super-diagonal 0/1 stationary into PSUM (halo_dn[m] =
up[m+1, row0]).  ACT drains PSUM to SBUF fp16 (DVE reading PSUM directly
measured ~10x slow); the edge adds then run on fp16 in SBUF.

Per-step engine split: DVE runs all 8 products, the up/um trees, the
interior combine adds and the interior-first/edges-last normalization
(GPSIMD is banned from the datapath: its tensor ucode runs ~3x below DVE
AND starves DVE's SBUF ports ~4x while active -- measured); the u0 class
never materializes a tree sum (its two column-shifted product planes
feed the PSUM groups and interior adds directly); ACT does the 2 PSUM
drains per combine; PE 16 matmuls per combine (halo + edge-row folding,
u0/um-last accumulation order so only the final matmul + drain sit on
the tail).  Setup streams loads through 4 rotating stage buffers
(depth-4 pipeline ~ HBM bound), casts on ACT (k<4) / DVE (k>=4), abs on
ACT, and runs the absw tree AND all of step 1's products/trees inside
the load window; gate-sum reuses the A-set tiles post-load.
"""

import sys

sys.path.insert(0, "/opt/trn_rl_repo")

import numpy as np

import concourse.bass as bass
import concourse.mybir as mybir
from concourse import tile
from concourse.bass_utils import run_bass_kernel_spmd

N_CORES = 8
B, K, H, W = 16, 8, 512, 512
BPC = B // N_CORES  # images per core (merged: 8 rows per partition)
P = 128
RPP = H // P  # rows per partition per image
R2 = BPC * RPP  # flattened rows per partition
WG = W + 4  # guarded row width (image cols at [2:514])
C0 = 2
PROP_TIME = 4

F32 = mybir.dt.float32
DT = mybir.dt.float16
AT = mybir.AluOpType
AF = mybir.ActivationFunctionType


def _split_excess_waits(nc):
    """This walrus build encodes at most 1 sem wait per instruction; move the
    overflow onto preceding NoOps. Also drop EVENT_SEMAPHORE_RANGE_CLEAR
    (unencodable here; only appears at the kernel tail where it's a no-op)."""
    for f in nc.m.functions:
        for bb in f.blocks:
            new_insts = []
            for ins in bb.instructions:
                if getattr(ins, "op_name", None) == "EVENT_SEMAPHORE_RANGE_CLEAR":
                    continue
                cap = 1
                si = getattr(ins, "sync_info", None)
                if si is not None and si.on_wait and len(si.on_wait) > cap:
                    extra = list(si.on_wait[cap:])
                    del si.on_wait[cap:]
                    while extra:
                        nop = mybir.InstNoOp(
                            name=nc.get_next_instruction_name(),
                            engine=ins.engine,
                            sync_info=mybir.SyncInfo(on_wait=extra[:cap], on_update=[]),
                        )
                        new_insts.append(nop)
                        extra = extra[cap:]
                new_insts.append(ins)
            bb.instructions[:] = new_insts


def _c(ap):
    """center (image) view of a guarded [P, R2, WG] tile."""
    return ap[:, :, C0 : C0 + W]


def _w(ap, dj):
    """column-shifted view of a guarded tile: value at [i, j+dj]."""
    return ap[:, :, C0 + dj : C0 + dj + W]


def _emit(nc, pool, psum, g_dram, d_dram, o_dram):
    V = nc.vector
    GP = nc.gpsimd
    ACT = nc.scalar
    PE = nc.tensor

    def gtile(name):  # guarded work tile
        return pool.tile([P, R2, WG], DT, name=name)

    def utile(name):  # unguarded work tile
        return pool.tile([P, R2, W], DT, name=name)

    gates = pool.tile([P, K, R2, WG], DT, name="gates")
    stages = [pool.tile([P, RPP, W], F32, name=f"stg{i}") for i in range(4)]
    # sA/sB: abs ping-pong (setup); tmp1/tmp2: product temps (step-1 stream
    # during the load window, then every prop step -- each product is
    # consumed by the immediately following tree add, so two suffice).
    sA, sB = gtile("sA"), gtile("sB")
    tmp1, tmp2 = gtile("tmp1"), gtile("tmp2")
    # A-set: absw trees (streamed), then gate-sum trees, then step 2..4 trees.
    upA, u0A, umA = utile("upA"), utile("u0A"), utile("umA")
    # C-set: step-1 class sums (streamed during loads); step 2..4 combine dest.
    upC, u0C, umC = utile("upC"), utile("u0C"), utile("umC")
    rA = gtile("rA")
    inv = utile("inv")
    bias = utile("bias")
    # shift matrices for the PE halo: halo_dn[m]=x[m+1], halo_up[m]=x[m-1];
    # wid = identity (accumulates in-partition rows into the same PSUM bank)
    wdn = pool.tile([P, P], DT, name="wdn")
    wup = pool.tile([P, P], DT, name="wup")
    wid = pool.tile([P, P], DT, name="wid")
    ci = pool.tile([P, P], F32, name="ci")
    pm1 = pool.tile([P, 1], F32, name="pm1")
    pp1 = pool.tile([P, 1], F32, name="pp1")
    pz0 = pool.tile([P, 1], F32, name="pz0")
    psum_dn = psum.tile([P, BPC, W], F32, name="psum_dn", bufs=2)
    psum_up = psum.tile([P, BPC, W], F32, name="psum_up", bufs=2)

    def gv(k, dj=0):  # column-shifted gate view [P, R2, W]
        return gates[:, k, :, C0 + dj : C0 + dj + W]

    # ---- loads: depth-4 pipeline over 4 rotating stage buffers; the first
    # triggers are emitted before any constants so DMA starts immediately ----
    load_i = 0

    def load(dram_plane, cast_eng, dst):
        nonlocal load_i
        st = stages[load_i % 4]
        load_i += 1
        nc.sync.dma_start(out=st[:], in_=dram_plane.rearrange("(p r) j -> p r j", p=P))
        if cast_eng is V:
            V.tensor_copy(dst, st[:])
        else:
            ACT.activation(dst, st[:], AF.Copy)

    for b in range(BPC):
        load(d_dram[b, 0], V, rA[:, 4 * b : 4 * b + 4, C0 : C0 + W])

    # ---- constants: shift matrices via iota + is_equal ----
    GP.iota(ci[:], [[1, P]], base=0, channel_multiplier=0,
            allow_small_or_imprecise_dtypes=True)  # ci[p,j] = j
    GP.iota(pm1[:], [[1, 1]], base=-1, channel_multiplier=1,
            allow_small_or_imprecise_dtypes=True)  # p-1
    GP.iota(pp1[:], [[1, 1]], base=1, channel_multiplier=1,
            allow_small_or_imprecise_dtypes=True)  # p+1
    GP.iota(pz0[:], [[1, 1]], base=0, channel_multiplier=1,
            allow_small_or_imprecise_dtypes=True)  # p
    # wdn[p,m] = 1 iff p == m+1  <=>  m == p-1 ; wup[p,m] = 1 iff m == p+1
    V.tensor_scalar(wdn[:], ci[:], pm1[:, 0:1], None, AT.is_equal)
    V.tensor_scalar(wup[:], ci[:], pp1[:, 0:1], None, AT.is_equal)
    V.tensor_scalar(wid[:], ci[:], pz0[:, 0:1], None, AT.is_equal)

    # ---- zero guard columns (written once; ops below write centers only) ----
    GP.memset(gates[:, :, :, 0:C0], 0.0)
    GP.memset(gates[:, :, :, C0 + W : WG], 0.0)
    for t in (sA, sB, tmp1, tmp2, rA):
        GP.memset(t[:, :, 0:C0], 0.0)
        GP.memset(t[:, :, C0 + W : WG], 0.0)

    # abs ping-pong: even k -> sA, odd k -> sB (product scratches, free here)
    def _abs(k):
        dst = sA if k % 2 == 0 else sB
        ACT.activation(_c(dst), gv(k), AF.Abs)
        return dst

    # Streamed per gate k: loads + cast, |g_k| (ACT), step-1 product
    # g_k * raw (DVE, into tmp1/tmp2), and both the absw tree (A-set) and
    # step-1 tree (C-set) as their operands complete.
    for k in range(K):
        eng = ACT if k < 4 else V
        for b in range(BPC):
            load(g_dram[b, k], eng, gates[:, k, 4 * b : 4 * b + 4, C0 : C0 + W])
        _abs(k)
        t = tmp1 if k % 2 == 0 else tmp2
        V.tensor_mul(_c(t), gv(k), _c(rA))
        if k == 1:
            V.tensor_add(upC[:], _w(tmp1, 1), _c(tmp2))
            V.tensor_add(upA[:], _w(sA, 1), _c(sB))
        elif k == 2:
            V.tensor_add(upC[:], upC[:], _w(tmp1, -1))
            V.tensor_add(upA[:], upA[:], _w(sA, -1))
        elif k == 4:
            V.tensor_add(u0C[:], _w(tmp2, 1), _w(tmp1, -1))
            V.tensor_add(u0A[:], _w(sB, 1), _w(sA, -1))
        elif k == 6:
            V.tensor_add(umC[:], _w(tmp2, 1), _c(tmp1))
            V.tensor_add(umA[:], _w(sB, 1), _c(sA))
        elif k == 7:
            V.tensor_add(umC[:], umC[:], _w(tmp2, -1))
            V.tensor_add(umA[:], umA[:], _w(sB, -1))

    def combine(dst, up_t, u0_t, um_t, u0_after_um=False):
        """dst[q] = up[q+1] + u0[q] + um[q-1] per image (q in 0..3).
        u0_t: a plane, or a list of fns mapping a row slice to a
        column-shifted view (the u0 class folded into the PSUM groups and
        interior adds without materializing its own tree sum).

        Partition-edge rows (q=0 and q=3) are built ENTIRELY in PSUM: the
        cross-partition halo term via the wdn/wup shift matmul plus the two
        in-partition terms via identity matmuls accumulating into the same
        bank; ACT drains then write those rows of dst directly.  DVE adds
        only the interior rows (q=1,2).  GPSIMD is never used: its tensor
        ucode both runs ~3x below DVE and starves DVE's SBUF ports ~4x
        while active (measured), so everything elementwise stays on DVE.
        """
        u0_fns = u0_t if isinstance(u0_t, list) else [lambda rs, t=u0_t: t[:, rs, :]]
        for b in range(BPC):
            mms = [(wdn, up_t[:, 4 * b, :])]
            mms += [(wid, f(4 * b + 3)) for f in u0_fns]
            mms += [(wid, um_t[:, 4 * b + 2, :])]
            if u0_after_um:  # u0 operands finish last: keep them at the tail
                mms = [mms[0], mms[-1]] + mms[1:-1]
            for i, (wm, rhs) in enumerate(mms):
                PE.matmul(out=psum_dn[:, b, :], lhsT=wm[:], rhs=rhs,
                          start=(i == 0), stop=(i == len(mms) - 1))
            mms = [(wid, up_t[:, 4 * b + 1, :])]
            mms += [(wid, f(4 * b)) for f in u0_fns]
            mms += [(wup, um_t[:, 4 * b + 3, :])]
            if u0_after_um:
                mms = [mms[0], mms[-1]] + mms[1:-1]
            for i, (wm, rhs) in enumerate(mms):
                PE.matmul(out=psum_up[:, b, :], lhsT=wm[:], rhs=rhs,
                          start=(i == 0), stop=(i == len(mms) - 1))
        ACT.activation(dst[:, 3::RPP, :], psum_dn[:], AF.Copy)
        ACT.activation(dst[:, 0::RPP, :], psum_up[:], AF.Copy)
        for b in range(BPC):
            V.tensor_add(dst[:, 4 * b + 1 : 4 * b + 3, :],
                         up_t[:, 4 * b + 2 : 4 * b + 4, :],
                         u0_fns[0](slice(4 * b + 1, 4 * b + 3)))
            for f in u0_fns[1:]:
                V.tensor_add(dst[:, 4 * b + 1 : 4 * b + 3, :],
                             dst[:, 4 * b + 1 : 4 * b + 3, :],
                             f(slice(4 * b + 1, 4 * b + 3)))
        for b in range(BPC):
            V.tensor_add(dst[:, 4 * b + 1 : 4 * b + 3, :],
                         dst[:, 4 * b + 1 : 4 * b + 3, :],
                         um_t[:, 4 * b : 4 * b + 2, :])

    # ---- absw -> inv = exp(-ln(absw)); combine lands in `bias` scratch ----
    combine(bias, upA, u0A, umA)
    for b in range(BPC):
        ACT.activation(stages[b][:], bias[:, 4 * b : 4 * b + 4, :], AF.Ln)
        ACT.activation(inv[:, 4 * b : 4 * b + 4, :], stages[b][:], AF.Exp, scale=-1.0)

    # ---- step-1 shift-sum (inputs streamed above); lands in tmp1 center ----
    combine(_c(tmp1), upC, u0C, umC)

    # ---- gate_sum (A-set reused) -> bias = raw - gate_sum*inv*raw;
    # the u0 class reads the (guarded) gate planes directly ----
    V.tensor_add(upA[:], gv(0, 1), gv(1))
    V.tensor_add(upA[:], upA[:], gv(2, -1))
    V.tensor_add(umA[:], gv(5, 1), gv(6))
    V.tensor_add(umA[:], umA[:], gv(7, -1))

    def gvr(k, dj):
        return lambda rs: gates[:, k, rs, C0 + dj : C0 + dj + W]

    def tvr(t, dj):
        return lambda rs: t[:, rs, C0 + dj : C0 + dj + W]

    combine(_c(tmp2), upA, [gvr(3, 1), gvr(4, -1)], umA)
    V.tensor_mul(_c(tmp2), _c(tmp2), _c(rA))  # gate_sum_unnorm * raw
    V.tensor_mul(_c(tmp2), _c(tmp2), inv[:])
    V.tensor_sub(bias[:], _c(rA), _c(tmp2))

    # ---- finish step 1: r1 = inv * U1 + bias ----
    V.tensor_mul(_c(tmp1), _c(tmp1), inv[:])
    V.tensor_add(_c(rA), _c(tmp1), bias[:])

    def norm_split(acc, last):
        """r = inv*acc + bias, interior rows first (edge rows arrive late
        via PE+drain), then the stepped edge rows."""
        for b in range(BPC):
            V.tensor_mul(acc[:, 4 * b + 1 : 4 * b + 3, :],
                         acc[:, 4 * b + 1 : 4 * b + 3, :],
                         inv[:, 4 * b + 1 : 4 * b + 3, :])
            V.tensor_add(rA[:, 4 * b + 1 : 4 * b + 3, C0 : C0 + W],
                         acc[:, 4 * b + 1 : 4 * b + 3, :],
                         bias[:, 4 * b + 1 : 4 * b + 3, :])
        for q0 in (3, 0):
            V.tensor_mul(acc[:, q0::RPP, :], acc[:, q0::RPP, :], inv[:, q0::RPP, :])
            V.tensor_add(rA[:, q0::RPP, C0 : C0 + W], acc[:, q0::RPP, :],
                         bias[:, q0::RPP, :])
        if last:
            for b in range(BPC):
                st = stages[b]
                V.tensor_copy(st[:], rA[:, 4 * b : 4 * b + 4, C0 : C0 + W])
                nc.sync.dma_start(
                    out=o_dram[b, 0].rearrange("(p r) j -> p r j", p=P), in_=st[:]
                )

    def norm_split_last(acc):
        """Final step: finish image b completely, then cast+store it while
        the other image's rows are still being normalized."""
        for b in range(BPC):
            V.tensor_mul(acc[:, 4 * b + 1 : 4 * b + 3, :],
                         acc[:, 4 * b + 1 : 4 * b + 3, :],
                         inv[:, 4 * b + 1 : 4 * b + 3, :])
            V.tensor_add(rA[:, 4 * b + 1 : 4 * b + 3, C0 : C0 + W],
                         acc[:, 4 * b + 1 : 4 * b + 3, :],
                         bias[:, 4 * b + 1 : 4 * b + 3, :])
            for q in (4 * b + 3, 4 * b):
                V.tensor_mul(acc[:, q, :], acc[:, q, :], inv[:, q, :])
                V.tensor_add(rA[:, q, C0 : C0 + W], acc[:, q, :], bias[:, q, :])
            # r4 is already fp16; store it directly (fp16 DRAM out, host
            # upcasts exactly) -- saves the cast ops and half the out DMA.
            nc.sync.dma_start(
                out=o_dram[b, 0].rearrange("(p r) j -> p r j", p=P),
                in_=rA[:, 4 * b : 4 * b + 4, C0 : C0 + W],
            )

    # ---- steps 2..4, r updated in place in rA ----
    # Class order up, um, then the two u0 products LAST (p3 -> tmp2, p4 ->
    # tmp1, both persisting through the combine): the u0 class never
    # materializes a tree sum -- its two column-shifted product planes feed
    # the PSUM groups and interior adds directly.
    for step in range(1, PROP_TIME):
        V.tensor_mul(_c(tmp1), gv(0), _c(rA))
        V.tensor_mul(_c(tmp2), gv(1), _c(rA))
        V.tensor_add(upA[:], _w(tmp1, 1), _c(tmp2))
        V.tensor_mul(_c(tmp1), gv(2), _c(rA))
        V.tensor_add(upA[:], upA[:], _w(tmp1, -1))
        V.tensor_mul(_c(tmp1), gv(5), _c(rA))
        V.tensor_mul(_c(tmp2), gv(6), _c(rA))
        V.tensor_add(umA[:], _w(tmp1, 1), _c(tmp2))
        V.tensor_mul(_c(tmp1), gv(7), _c(rA))
        V.tensor_add(umA[:], umA[:], _w(tmp1, -1))
        V.tensor_mul(_c(tmp2), gv(3), _c(rA))
        V.tensor_mul(_c(tmp1), gv(4), _c(rA))
        combine(upC, upA, [tvr(tmp2, 1), tvr(tmp1, -1)], umA, u0_after_um=True)
        if step < PROP_TIME - 1:
            norm_split(upC, last=False)
        else:
            norm_split_last(upC)


def build(legalize=True):
    nc = bass.Bass()
    g_dram = nc.declare_dram_parameter("guidance", [BPC, K, H, W], F32, isOutput=False)
    d_dram = nc.declare_dram_parameter("blur_depth", [BPC, 1, H, W], F32, isOutput=False)
    o_dram = nc.declare_dram_parameter("out", [BPC, 1, H, W], DT, isOutput=True)
    with tile.TileContext(nc) as tc:
        with tc.tile_pool(name="main", bufs=1) as pool:
            with tc.tile_pool(name="ps", space="PSUM", bufs=1) as psum:
                _emit(nc, pool, psum, g_dram, d_dram, o_dram)
    if legalize:
        _split_excess_waits(nc)
    return nc


_NC = None


def _get_nc():
    global _NC
    if _NC is None:
        _NC = build()
    return _NC


def run(guidance, blur_depth, **spmd_kwargs):
    nc = _get_nc()
    in_maps = [
        {
            "guidance": np.ascontiguousarray(guidance[BPC * c : BPC * (c + 1)]),
            "blur_depth": np.ascontiguousarray(blur_depth[BPC * c : BPC * (c + 1)]),
        }
        for c in range(N_CORES)
    ]
    res = run_bass_kernel_spmd(nc, in_maps, list(range(N_CORES)), **spmd_kwargs)
    out = np.concatenate([res.results[i]["out"] for i in range(N_CORES)], axis=0)
    return out, res


def kernel(guidance, blur_depth):
    out, _ = run(guidance, blur_depth)
    return out.astype(np.float32)
